# revision 46
# baseline (speedup 1.0000x reference)
"""MultiHeadAttention Trainium2 kernel (8 NeuronCores, SPMD, no collectives).

Reference model: B=4, S=2048, D=1024, H=16, Dh=64.
  q/k/v = split_heads(x @ W.T + b); scores = q k^T / sqrt(Dh); mask==0 -> -1e9;
  softmax; out = (attn v) @ fc_w.T + fc_b.

Sharding: core c handles batch b=c//2 and query rows [1024*(c%2), +1024).
K/V projections are recomputed per query-half which avoids any collective:
each core writes a disjoint [1024, 1024] output slice.

v2 schedule (vs v1's phase-serial layout):
  - Score matmuls for the head pair (2j, 2j+1) are emitted adjacently; their
    operands live at base partitions 0/64, so the PE runs them concurrently
    in distinct row-groups (the K=64 contraction only half-fills the array).
  - K/Q projections for j-blocks 1..7 are interleaved into the attention
    pair loop, filling the PE stalls left by the exp->PV dependency and
    keeping the HAM clock-gate warm.
  - One shared 2-buffer PSUM "work" pool rotates between projection
    accumulation groups, score tiles, and recip-broadcasts; PV keeps 4
    banks (65-row tiles; a trailing ones-column in V emits softmax
    row-sums as PV row 64).
  - K^T/Q^T live in per-j streaming tiles (bufs=4) instead of persistent
    [P,8,S] tensors; fc weights DMA late into recycled x-tile slots.
"""

import os

import numpy as np
import ml_dtypes

BF16 = ml_dtypes.bfloat16

D = 1024
S = 2048
B = 4
H = 16
DH = 64
TQ = 1024  # query rows per core
P = 128
N_CORES = 8

_CACHED = {}


def _build():
    import concourse.bass as bass
    import concourse.mybir as mybir
    import concourse.tile as tile
    from concourse import bacc

    BF = mybir.dt.bfloat16
    F32 = mybir.dt.float32
    F32R = mybir.dt.float32r
    AF = mybir.ActivationFunctionType

    nc = bacc.Bacc("TRN2", target_bir_lowering=False, debug=False)

    xqT = nc.dram_tensor("xqT", [D, TQ], BF, kind="ExternalInput").ap()
    xkT = nc.dram_tensor("xkT", [D, S], BF, kind="ExternalInput").ap()
    xvT = nc.dram_tensor("xvT", [D, S], BF, kind="ExternalInput").ap()
    wqT = nc.dram_tensor("wqT", [D, D], BF, kind="ExternalInput").ap()
    wkT = nc.dram_tensor("wkT", [D, D], BF, kind="ExternalInput").ap()
    wvT = nc.dram_tensor("wvT", [D, D], BF, kind="ExternalInput").ap()
    fcT = nc.dram_tensor("fcT", [D, D], BF, kind="ExternalInput").ap()
    maskT = nc.dram_tensor("maskT", [S, TQ], BF, kind="ExternalInput").ap()
    bq_d = nc.dram_tensor("bq", [P, 8], F32, kind="ExternalInput").ap()
    bk_d = nc.dram_tensor("bk", [P, 8], F32, kind="ExternalInput").ap()
    fcb_d = nc.dram_tensor("fcb", [1, D], BF, kind="ExternalInput").ap()
    out = nc.dram_tensor("out", [TQ, D], F32, kind="ExternalOutput").ap()

    VGW = 66  # per-head group width in V storage: 64 V cols + ones col + pad
    VGPAD = 62  # tail pad so 128-wide lhsT windows stay in bounds
    NT = S // P  # 16 key tiles

    with tile.TileContext(nc) as tc:
        with (
            tc.tile_pool(name="const", bufs=1) as const,
            tc.tile_pool(name="work", bufs=2, space="PSUM") as work,
            tc.tile_pool(name="xkp", bufs=8) as xkp,
            tc.tile_pool(name="xqp", bufs=8) as xqp,
            tc.tile_pool(name="wvp", bufs=1) as wvp,
            tc.tile_pool(name="xvp", bufs=3) as xvp,
            tc.tile_pool(name="wkq", bufs=3) as wkq,
            tc.tile_pool(name="ktq", bufs=4) as ktq,
            tc.tile_pool(name="ptp", bufs=4) as ptp,
            tc.tile_pool(name="nrm", bufs=1) as nrm,
        ):
            VG = const.tile([P, NT, H * VGW + VGPAD], BF)  # V (+ones), tk-tiled
            AOT = const.tile([P, 8, TQ], BF)       # attn-out^T (d_out, tq)
            MSK = const.tile([P, NT, TQ], BF)      # mask^T, tk-tiled
            bq_s = const.tile([P, 8], F32)
            bk_s = const.tile([P, 8], F32)
            fcb_s = const.tile([1, D], BF)
            fcb_bc = const.tile([P, D], BF)

            nc.sync.dma_start(bq_s[:], bq_d)
            nc.sync.dma_start(bk_s[:], bk_d)
            nc.sync.dma_start(fcb_s[:], fcb_d)
            nc.gpsimd.partition_broadcast(fcb_bc[:], fcb_s[:])
            for tt in range(NT):
                nc.vector.memset(VG[:, tt], 1.0)

            wk_j = {}
            wq_j = {}

            def fetch_w(j):
                tk_ = wkq.tile([P, 8, P], BF, tag="wk", name=f"wk{j}")
                nc.sync.dma_start(
                    tk_[:],
                    wkT[:, j * P:(j + 1) * P].rearrange("(i p) n -> p i n", p=P),
                )
                wk_j[j] = tk_
                tq_ = wkq.tile([P, 8, P], BF, tag="wq", name=f"wq{j}")
                nc.sync.dma_start(
                    tq_[:],
                    wqT[:, j * P:(j + 1) * P].rearrange("(i p) n -> p i n", p=P),
                )
                wq_j[j] = tq_

            # DMA order = need order: q-side first (qproj runs first),
            # then wv + early xv (V tiles), then k-side, then the rest
            fetch_w(0)
            xq = []
            for di in range(8):
                t_ = xqp.tile([P, TQ], BF, tag="xq")
                nc.sync.dma_start(t_[:], xqT[di * P:(di + 1) * P, :])
                xq.append(t_)
            xk = []
            for di in range(8):
                t_ = xkp.tile([P, S], BF, tag="xt")
                nc.sync.dma_start(t_[:], xkT[di * P:(di + 1) * P, :])
                xk.append(t_)
            wv_s = wvp.tile([P, 8, D], BF, tag="w")
            nc.sync.dma_start(
                wv_s[:, 0:4], wvT[0:512, :].rearrange("(j p) n -> p j n", p=P)
            )
            nc.sync.dma_start(
                wv_s[:, 4:8], wvT[512:1024, :].rearrange("(j p) n -> p j n", p=P)
            )
            xv = {}

            def fetch_xv(tt):
                t_ = xvp.tile([P, 8, P], BF, tag="xv", name=f"xv{tt}")
                nc.sync.dma_start(
                    t_[:],
                    xvT[:, tt * P:(tt + 1) * P].rearrange("(j p) n -> p j n", p=P),
                )
                xv[tt] = t_

            for tt in range(3):
                fetch_xv(tt)

            nc.sync.dma_start(MSK[:], maskT.rearrange("(t p) q -> p t q", p=P))
            for jw in range(1, 3):
                fetch_w(jw)

            def vgroup(tt):
                # V[t, do] = sum_di xvT[di,t] * wvT[di,do] for one t tile
                if tt + 3 <= NT - 1:
                    fetch_xv(tt + 3)
                vp = work.tile([P, TQ], F32, tag="wk")
                for di in range(8):
                    for n in range(2):
                        nc.tensor.matmul(
                            vp[:, n * 512:(n + 1) * 512],
                            lhsT=xv[tt][:, di, :],
                            rhs=wv_s[:, di, n * 512:(n + 1) * 512],
                            start=(di == 0),
                            stop=(di == 7),
                        )
                nc.vector.tensor_copy(
                    VG[:, tt, 0:H * VGW].rearrange(
                        "p (h c) -> p h c", c=VGW
                    )[:, :, :64],
                    vp.rearrange("p (h c) -> p h c", c=64),
                )

            KT_j = {}
            QT_j = {}

            def kproj_q(j, q4):
                # one 512-wide chunk of K^T_j: 8 di-matmuls + evac
                if j not in KT_j:
                    KT_j[j] = ktq.tile([P, S], BF, tag="KT", name=f"KT{j}")
                kp = work.tile([P, 512], F32, tag="wk")
                for di in range(8):
                    nc.tensor.matmul(
                        kp[:],
                        lhsT=wk_j[j][:, di, :],
                        rhs=xk[di][:, q4 * 512:(q4 + 1) * 512],
                        start=(di == 0),
                        stop=(di == 7),
                    )
                nc.vector.tensor_scalar_add(
                    KT_j[j][:, q4 * 512:(q4 + 1) * 512],
                    kp[:],
                    bk_s[:, j:j + 1],
                )

            def kproj_half(j, half):
                kproj_q(j, 2 * half)
                kproj_q(j, 2 * half + 1)

            def qproj_q(j, n2):
                if j not in QT_j:
                    QT_j[j] = ktq.tile(
                        [P, TQ], BF, tag="QT", name=f"QT{j}", bufs=3
                    )
                qp = work.tile([P, 512], F32, tag="wk")
                for di in range(8):
                    nc.tensor.matmul(
                        qp[:],
                        lhsT=wq_j[j][:, di, :],
                        rhs=xq[di][:, n2 * 512:(n2 + 1) * 512],
                        start=(di == 0),
                        stop=(di == 7),
                    )
                nc.vector.tensor_scalar_add(
                    QT_j[j][:, n2 * 512:(n2 + 1) * 512],
                    qp[:],
                    bq_s[:, j:j + 1],
                )

            def qproj(j):
                qproj_q(j, 0)
                qproj_q(j, 1)

            qproj(0)
            kproj_half(0, 0)
            kproj_half(0, 1)
            vgroup(0)
            vgroup(1)

            # projection groups for j=1..7, interleaved into pairs >= 1
            backlog = []
            for j in range(1, 8):
                for q4 in range(4):
                    backlog.append(lambda j=j, q4=q4: kproj_q(j, q4))
                backlog.append(lambda j=j: qproj_q(j, 0))
                backlog.append(lambda j=j: qproj_q(j, 1))
            bi = 0

            # fc weights + bf16 fc-partial accumulators in recycled x slots
            fct = []
            fcpart = []

            def alloc_fc_tiles():
                for g in range(4):
                    t_ = xkp.tile([P, S], BF, tag="xt", name=f"fct{g}")
                    nc.sync.dma_start(
                        t_.rearrange("p (jj n) -> p jj n", n=D),
                        fcT[g * 256:(g + 1) * 256, :].rearrange(
                            "(jj p) n -> p jj n", p=P
                        ),
                    )
                    fct.append(t_)
                for g in range(4):
                    t_ = xkp.tile([P, S], BF, tag="xt", name=f"fcpart{g}")
                    fcpart.append(t_)

            def fc_half1(tt):
                fp = work.tile([P, TQ], F32, tag="wk")
                for jj in range(4):
                    for n in range(2):
                        nc.tensor.matmul(
                            fp[:, n * 512:(n + 1) * 512],
                            lhsT=AOT[:, jj, tt * P:(tt + 1) * P],
                            rhs=fct[jj // 2][
                                :,
                                (jj % 2) * D + n * 512:
                                (jj % 2) * D + (n + 1) * 512,
                            ],
                            start=(jj == 0),
                            stop=(jj == 3),
                        )
                nc.vector.tensor_copy(
                    fcpart[tt // 2].rearrange("p (t n) -> p t n", n=D)[
                        :, tt % 2
                    ],
                    fp[:],
                )

            # ---------------- attention (head pairs) ----------------
            with tc.tile_pool(name="pvp", bufs=4, space="PSUM") as pvp:
                for p in range(8):
                    j = p
                    h0, h1 = 2 * p, 2 * p + 1
                    if 3 <= p + 3 < 8:
                        fetch_w(p + 3)
                    if p == 5:
                        alloc_fc_tiles()
                        for tt in range(8):
                            backlog.append(lambda tt=tt: fc_half1(tt))
                    kt = KT_j[j]
                    qt = QT_j[j]
                    pv0 = [
                        pvp.tile([P, 512], F32, tag="pv", name=f"pv0_{c}")
                        for c in range(2)
                    ]
                    pv1 = [
                        pvp.tile([P, 512], F32, tag="pv", name=f"pv1_{c}")
                        for c in range(2)
                    ]
                    LAG = 2
                    pts = {}

                    def emit_pv(tk):
                        # lhsT is a 128-wide window: cols 0..63 = V_h, col
                        # 64 = ones (softmax row-sum lands in psum row 64),
                        # cols 65..127 spill into the neighbors -> psum
                        # rows 65..127 are garbage and never read
                        qa, qb = pts.pop(tk)
                        for c, q in ((0, qa), (1, qb)):
                            nc.tensor.matmul(
                                pv0[c][:],
                                lhsT=VG[:, tk, h0 * VGW:h0 * VGW + P],
                                rhs=q[:, 0:512],
                                start=(tk == 0),
                                stop=(tk == NT - 1),
                            )
                            nc.tensor.matmul(
                                pv1[c][:],
                                lhsT=VG[:, tk, h1 * VGW:h1 * VGW + P],
                                rhs=q[:, 512:1024],
                                start=(tk == 0),
                                stop=(tk == NT - 1),
                            )

                    for tk in range(NT + LAG):
                        if tk < NT:
                            # chunk-major score tiles: both heads of the
                            # pair at base partitions 0/64 -> adjacent
                            # matmuls run in distinct PE row-groups
                            sa = work.tile([P, TQ], F32, tag="wk")
                            sb = work.tile([P, TQ], F32, tag="wk")
                            for s, c in ((sa, 0), (sb, 1)):
                                nc.tensor.matmul(
                                    s[:, 0:512],
                                    lhsT=kt[0:64, tk * P:(tk + 1) * P],
                                    rhs=qt[0:64, c * 512:(c + 1) * 512],
                                    start=True,
                                    stop=True,
                                )
                                nc.tensor.matmul(
                                    s[:, 512:1024],
                                    lhsT=kt[64:128, tk * P:(tk + 1) * P],
                                    rhs=qt[64:128, c * 512:(c + 1) * 512],
                                    start=True,
                                    stop=True,
                                )
                            pta = ptp.tile([P, TQ], BF, tag="pt")
                            nc.scalar.activation(
                                pta[:], sa[:], AF.Exp, scale=0.125
                            )
                            ptb = ptp.tile([P, TQ], BF, tag="pt")
                            nc.scalar.activation(
                                ptb[:], sb[:], AF.Exp, scale=0.125
                            )
                            for c, pt_ in ((0, pta), (1, ptb)):
                                m_ = MSK[:, tk, c * 512:(c + 1) * 512]
                                nc.vector.tensor_mul(
                                    pt_[:, 0:512], pt_[:, 0:512], m_
                                )
                                nc.vector.tensor_mul(
                                    pt_[:, 512:1024], pt_[:, 512:1024], m_
                                )
                            pts[tk] = (pta, ptb)
                        if tk >= LAG:
                            emit_pv(tk - LAG)
                        # filler PE work after this iteration's scores, so
                        # the scalar engine always has fresh psum to exp
                        if p == 0 and tk + 2 <= NT - 1:
                            vgroup(tk + 2)
                        if ((p == 0 and tk >= 12)
                                or (p >= 1 and tk in (1, 3, 5, 6, 7, 9, 11, 13))
                                or (p >= 6 and tk in (16, 17))) and bi < len(backlog):
                            backlog[bi]()
                            bi += 1
                    # normalize both heads (PE-free; V bias folded into the
                    # fc bias host-side)
                    for h, pv in ((h0, pv0), (h1, pv1)):
                        bp = 64 * (h % 2)
                        rs = nrm.tile([1, TQ], F32, tag="rs")
                        for c in range(2):
                            nc.vector.tensor_copy(
                                rs[:, c * 512:(c + 1) * 512], pv[c][64:65, :]
                            )
                        nc.vector.reciprocal_approx_fast(rs[:], rs[:])
                        bcs = nrm.tile([64, TQ], F32, tag="bcs", bufs=1)
                        nc.gpsimd.partition_broadcast(bcs[:], rs[:])
                        for c in range(2):
                            nc.vector.tensor_mul(
                                AOT[bp:bp + 64, j, c * 512:(c + 1) * 512],
                                pv[c][:64, :],
                                bcs[:, c * 512:(c + 1) * 512],
                            )

            # ---------------- output projection (j=4..7 + partial) ------
            with tc.tile_pool(name="fpsum", bufs=2, space="PSUM") as fpsum:
                for tt in range(8):
                    ps = [
                        fpsum.tile([P, 512], F32, tag="fp", name=f"fp{n}")
                        for n in range(2)
                    ]
                    for jj in range(4, 8):
                        for n in range(2):
                            nc.tensor.matmul(
                                ps[n][:],
                                lhsT=AOT[:, jj, tt * P:(tt + 1) * P],
                                rhs=fct[jj // 2][
                                    :,
                                    (jj % 2) * D + n * 512:
                                    (jj % 2) * D + (n + 1) * 512,
                                ],
                                start=(jj == 4),
                                stop=(jj == 7),
                            )
                    part = fcpart[tt // 2].rearrange(
                        "p (t n) -> p t n", n=D
                    )[:, tt % 2]
                    for n in range(2):
                        ob = xqp.tile([P, 512], F32, tag="xq", name=f"ob{n}")
                        nc.vector.tensor_add(
                            ob[:], ps[n][:],
                            part[:, n * 512:(n + 1) * 512],
                        )
                        nc.vector.tensor_add(
                            ob[:], ob[:],
                            fcb_bc[:, n * 512:(n + 1) * 512],
                        )
                        nc.sync.dma_start(
                            out[tt * P:(tt + 1) * P, n * 512:(n + 1) * 512],
                            ob[:],
                        )

    nc.compile()
    return nc


def _get_nc():
    if "nc" not in _CACHED:
        _CACHED["nc"] = _build()
    return _CACHED["nc"]


def kernel(**inputs):
    from concourse import bass_utils

    query = np.asarray(inputs["query"], np.float32)
    key_in = np.asarray(inputs["key_in"], np.float32)
    value = np.asarray(inputs["value"], np.float32)
    mask = np.asarray(inputs["mask"])
    wq_w = np.asarray(inputs["wq_w"], np.float32)
    wq_b = np.asarray(inputs["wq_b"], np.float32)
    wk_w = np.asarray(inputs["wk_w"], np.float32)
    wk_b = np.asarray(inputs["wk_b"], np.float32)
    wv_w = np.asarray(inputs["wv_w"], np.float32)
    wv_b = np.asarray(inputs["wv_b"], np.float32)
    fc_w = np.asarray(inputs["fc_w"], np.float32)
    fc_b = np.asarray(inputs["fc_b"], np.float32)

    def c(a):
        return np.ascontiguousarray(a)

    shared = {
        "wqT": c(wq_w.T.astype(BF16)),
        "wkT": c(wk_w.T.astype(BF16)),
        "wvT": c(wv_w.T.astype(BF16)),
        "fcT": c(fc_w.T.astype(BF16)),
        "bq": c(wq_b.reshape(8, P).T.astype(np.float32)),
        "bk": c(wk_b.reshape(8, P).T.astype(np.float32)),
        "fcb": c((fc_b + wv_b @ fc_w.T).reshape(1, D).astype(BF16)),
    }

    in_maps = []
    for core in range(N_CORES):
        b, q0 = core // 2, TQ * (core % 2)
        m = dict(shared)
        m["xqT"] = c(query[b].T[:, q0:q0 + TQ].astype(BF16))
        m["xkT"] = c(key_in[b].T.astype(BF16))
        m["xvT"] = c(value[b].T.astype(BF16))
        m["maskT"] = c(mask[b][q0:q0 + TQ, :].T.astype(BF16))
        in_maps.append(m)

    nc = _get_nc()
    trace = bool(int(os.environ.get("KERNEL_TRACE", "0")))
    res = bass_utils.run_bass_kernel_spmd(
        nc, in_maps, core_ids=list(range(N_CORES)), trace=trace,
        **({"trace_cores": [0]} if trace else {}),
    )
    _CACHED["last_results"] = res

    full = np.empty((B, S, D), np.float32)
    for core in range(N_CORES):
        b, q0 = core // 2, TQ * (core % 2)
        full[b, q0:q0 + TQ, :] = res.results[core]["out"]
    return full
